# revision 12
# baseline (speedup 1.0000x reference)
"""Complex-magnitude MaxPool2d (k=2, s=2) Trainium2 Bass kernel.

Input  x:  [16, 2, 64, 224, 224] f32  (plane 0 = real, plane 1 = imag)
Output:    [16, 2, 64, 112, 112] f32  (value of the window element with the
                                       largest |z|^2 = re^2 + im^2)

Sharding: pure data parallel over batch: 16 / 8 cores = 2 examples per core;
2(batch) x 64(channel) = 128 image planes map 1:1 onto SBUF partitions.

Host layout per row: [even-column (re,im) pairs | odd-column pairs]
([p, h, t, w2, ri]).  One contiguous 25KB-per-partition DMA per 14-row
chunk, and every heavy engine stream (masks, maxes, select data and
pre-fills) is contiguous; only the norm add reads stride-2 and the
copy_predicated masks broadcast step-0 over the (re,im) pair.

  ACT  : squares (one contiguous Square per chunk), select pre-fills
  DVE  : norm add, is_ge masks, horizontal max, copy_predicated selects
  DMA  : 16 input chunks (14 rows), 16 output stores (7 rows), input
         tile triple-buffered so the 9us chunk DMA stays 2 chunks ahead

Selection reproduces jnp.argmax's first-index tie-break (horizontal
is_ge: even/left wins; vertical is_ge: top wins); norm arithmetic is
fl(fl(re^2)+fl(im^2)), bit-exact with the reference.
"""

import numpy as np

import concourse.bass as bass
import concourse.mybir as mybir
from concourse import bacc, bass_utils, tile

NCORES = 8
B = 2            # batch per core
RI = 2           # real/imag
C = 64           # channels
H = W = 224
HO, WO = H // 2, W // 2
P = 128          # SBUF partitions = B * C
R = 14           # image rows per chunk (one DMA = one compute step)
NCHUNK = H // R  # 16
W2 = W // 2      # 112 column pairs
RP = R // 2      # 7 output rows per chunk

F32 = mybir.dt.float32
BF16 = mybir.dt.bfloat16
U8 = mybir.dt.uint8
OP = mybir.AluOpType
ACTF = mybir.ActivationFunctionType

# chunk row schedule: tiny leading chunks collapse the pipeline-fill ramp
# (a full 14-row chunk needs ~9us DMA + 5.5us of squares before the vector
# engine can start); sizes must be even and sum to H
CHUNKS = [4, 8, 12] + [14] * 14 + [4]
assert sum(CHUNKS) == H

_NC_CACHE = []


def _build_nc() -> bass.Bass:
    nc = bacc.Bacc("TRN2", target_bir_lowering=False, debug=False)
    x = nc.dram_tensor("x", [P, H, W * RI], F32, kind="ExternalInput").ap()
    out = nc.dram_tensor("out", [P, HO, WO * RI], F32, kind="ExternalOutput").ap()

    starts = [sum(CHUNKS[:j]) for j in range(len(CHUNKS))]
    LOOKAHEAD = 3  # input DMAs issued this many chunks ahead of the output
    # DMAs so the in-order trigger queue never stalls the input prefetch

    with tile.TileContext(nc) as tc:
        with tc.tile_pool(name="pool", bufs=2) as pool:
            xT = {}

            def load(j):
                if j >= len(CHUNKS):
                    return
                Rj = CHUNKS[j]
                xri = pool.tile(
                    [P, R * W * RI], F32, tag="xri", name="xri", bufs=LOOKAHEAD
                )
                xT[j] = xri
                nc.sync.dma_start(
                    out=xri[:, : Rj * W * RI].rearrange("p (r f) -> p r f", r=Rj),
                    in_=x[:, starts[j] : starts[j] + Rj, :],
                )

            for j in range(LOOKAHEAD):
                load(j)

            for k, Rk in enumerate(CHUNKS):
                r0 = starts[k]
                RPk = Rk // 2
                NH = Rk * W2             # horizontal windows per chunk
                NO = RPk * W2            # output windows per chunk
                NVAL = Rk * W * RI       # f32 values per chunk

                xri = xT.pop(k)[:, :NVAL]

                # squares of the whole chunk in one contiguous ACT op
                sq = pool.tile([P, R * W * RI], F32, tag="sq", name="sq")[:, :NVAL]
                nc.scalar.activation(out=sq, in_=xri, func=ACTF.Square)

                # norm2 = re^2 + im^2 (stride-2 reads, contiguous out)
                nrm = pool.tile([P, R * W2 * 2], F32, tag="nrm", name="nrm")[:, : NH * 2]
                sqp = sq.rearrange("p (n ri) -> p n ri", ri=RI)
                nc.vector.tensor_tensor(
                    out=nrm, in0=sqp[:, :, 0], in1=sqp[:, :, 1], op=OP.add
                )
                nE3 = nrm.rearrange("p (r t w2) -> p r t w2", r=Rk, t=2, w2=W2)
                nE, nO = nE3[:, :, 0, :], nE3[:, :, 1, :]

                # horizontal mask (even/left wins ties) + horizontal max.
                # masks are bf16 0/1.0: bitcast to u8 gives bytes (0x80,0x3f)
                # per true lane -- a pair-broadcast mask with no step-0 AP
                cH = pool.tile([P, R * W2], BF16, tag="cH", name="cH")[:, :NH]
                cH3 = cH.rearrange("p (r w2) -> p r w2", r=Rk, w2=W2)
                nc.vector.tensor_tensor(out=cH3, in0=nE, in1=nO, op=OP.is_ge)
                mH = pool.tile([P, R * W2], F32, tag="mH", name="mH")[:, :NH]
                mH3 = mH.rearrange("p (r w2) -> p r w2", r=Rk, w2=W2)
                nc.vector.tensor_tensor(out=mH3, in0=nE, in1=nO, op=OP.max)

                # horizontal select of (re,im) pairs: pre-fill odd-column
                # candidates (contiguous block), overwrite where even wins
                xp = xri.rearrange(
                    "p (r t w2 ri) -> p r t w2 ri", r=Rk, t=2, w2=W2, ri=RI
                )
                riH = pool.tile([P, R * W2 * RI], F32, tag="riH", name="riH")[:, : NH * RI]
                riH4 = riH.rearrange(
                    "p (r w2 ri) -> p r w2 ri", r=Rk, w2=W2, ri=RI
                )
                nc.scalar.copy(out=riH4, in_=xp[:, :, 1, :, :])
                nc.vector.copy_predicated(
                    out=riH4,
                    mask=cH.bitcast(U8).rearrange(
                        "p (r w2 ri) -> p r w2 ri", r=Rk, w2=W2, ri=RI
                    ),
                    data=xp[:, :, 0, :, :],
                )

                # vertical mask from horizontal maxes (top wins ties)
                mHr = mH.rearrange(
                    "p (rp rt w2) -> p rp rt w2", rp=RPk, rt=2, w2=W2
                )
                cV = pool.tile([P, RP * W2], BF16, tag="cV", name="cV")[:, :NO]
                cV3 = cV.rearrange("p (rp w2) -> p rp w2", rp=RPk, w2=W2)
                nc.vector.tensor_tensor(
                    out=cV3, in0=mHr[:, :, 0, :], in1=mHr[:, :, 1, :], op=OP.is_ge
                )

                # vertical select into the output tile
                riHr = riH.rearrange(
                    "p (rp rt w2 ri) -> p rp rt w2 ri", rp=RPk, rt=2, w2=W2, ri=RI
                )
                outT = pool.tile([P, RP * W2 * RI], F32, tag="outT", name="outT")[:, : NO * RI]
                outT4 = outT.rearrange(
                    "p (rp w2 ri) -> p rp w2 ri", rp=RPk, w2=W2, ri=RI
                )
                nc.scalar.copy(out=outT4, in_=riHr[:, :, 1, :, :])
                nc.vector.copy_predicated(
                    out=outT4,
                    mask=cV.bitcast(U8).rearrange(
                        "p (rp w2 ri) -> p rp w2 ri", rp=RPk, w2=W2, ri=RI
                    ),
                    data=riHr[:, :, 0, :, :],
                )

                load(k + LOOKAHEAD)
                nc.sync.dma_start(
                    out=out[:, r0 // 2 : r0 // 2 + RPk, :],
                    in_=outT.rearrange("p (rp f) -> p rp f", rp=RPk),
                )
    nc.compile()
    return nc


def get_nc() -> bass.Bass:
    if not _NC_CACHE:
        _NC_CACHE.append(_build_nc())
    return _NC_CACHE[0]


def kernel(x: np.ndarray, **run_kwargs) -> np.ndarray:
    nc = get_nc()
    xs = np.asarray(x, dtype=np.float32)
    assert xs.shape == (NCORES * B, RI, C, H, W), xs.shape
    # [16,2,64,H,W] -> [16,64,H,W2,t,2] -> row blocks [16,64,H,t,W2,2]
    xt = xs.transpose(0, 2, 3, 4, 1).reshape(NCORES * B, C, H, W2, 2, RI)
    xt = np.ascontiguousarray(xt.transpose(0, 1, 2, 4, 3, 5))
    in_maps = [
        {"x": xt[B * i : B * (i + 1)].reshape(P, H, W * RI)} for i in range(NCORES)
    ]
    res = bass_utils.run_bass_kernel_spmd(
        nc, in_maps, core_ids=list(range(NCORES)), **run_kwargs
    )
    # per-core [128, HO, WO*2] -> [b, c, HO, WO, ri] -> [b, ri, c, HO, WO]
    outs = [
        res.results[i]["out"].reshape(B, C, HO, WO, RI).transpose(0, 4, 1, 2, 3)
        for i in range(NCORES)
    ]
    out = np.concatenate(outs, axis=0)
    if run_kwargs:
        kernel.last_results = res
    return np.ascontiguousarray(out)


# revision 13
# speedup vs baseline: 1.0018x; 1.0018x over previous
"""Complex-magnitude MaxPool2d (k=2, s=2) Trainium2 Bass kernel.

Input  x:  [16, 2, 64, 224, 224] f32  (plane 0 = real, plane 1 = imag)
Output:    [16, 2, 64, 112, 112] f32  (value of the window element with the
                                       largest |z|^2 = re^2 + im^2)

Sharding: pure data parallel over batch: 16 / 8 cores = 2 examples per core;
2(batch) x 64(channel) = 128 image planes map 1:1 onto SBUF partitions.

Host layout per row: [even-column (re,im) pairs | odd-column pairs]
([p, h, t, w2, ri]).  One contiguous 25KB-per-partition DMA per 14-row
chunk, and every heavy engine stream (masks, maxes, select data and
pre-fills) is contiguous; only the norm add reads stride-2 and the
copy_predicated masks broadcast step-0 over the (re,im) pair.

  ACT  : squares (one contiguous Square per chunk), select pre-fills
  DVE  : norm add, is_ge masks, horizontal max, copy_predicated selects
  DMA  : 16 input chunks (14 rows), 16 output stores (7 rows), input
         tile triple-buffered so the 9us chunk DMA stays 2 chunks ahead

Selection reproduces jnp.argmax's first-index tie-break (horizontal
is_ge: even/left wins; vertical is_ge: top wins); norm arithmetic is
fl(fl(re^2)+fl(im^2)), bit-exact with the reference.
"""

import numpy as np

import concourse.bass as bass
import concourse.mybir as mybir
from concourse import bacc, bass_utils, tile

NCORES = 8
B = 2            # batch per core
RI = 2           # real/imag
C = 64           # channels
H = W = 224
HO, WO = H // 2, W // 2
P = 128          # SBUF partitions = B * C
R = 14           # image rows per chunk (one DMA = one compute step)
NCHUNK = H // R  # 16
W2 = W // 2      # 112 column pairs
RP = R // 2      # 7 output rows per chunk

F32 = mybir.dt.float32
BF16 = mybir.dt.bfloat16
U8 = mybir.dt.uint8
OP = mybir.AluOpType
ACTF = mybir.ActivationFunctionType

# chunk row schedule: tiny leading chunks collapse the pipeline-fill ramp
# (a full 14-row chunk needs ~9us DMA + 5.5us of squares before the vector
# engine can start); sizes must be even and sum to H
CHUNKS = [2, 4, 6, 8, 10, 12] + [14] * 12 + [8, 4, 2]
assert sum(CHUNKS) == H

_NC_CACHE = []


def _build_nc() -> bass.Bass:
    nc = bacc.Bacc("TRN2", target_bir_lowering=False, debug=False)
    x = nc.dram_tensor("x", [P, H, W * RI], F32, kind="ExternalInput").ap()
    out = nc.dram_tensor("out", [P, HO, WO * RI], F32, kind="ExternalOutput").ap()

    starts = [sum(CHUNKS[:j]) for j in range(len(CHUNKS))]
    LOOKAHEAD = 3  # input DMAs issued this many chunks ahead of the output
    # DMAs so the in-order trigger queue never stalls the input prefetch

    with tile.TileContext(nc) as tc:
        with tc.tile_pool(name="pool", bufs=2) as pool:
            xT = {}

            def load(j):
                if j >= len(CHUNKS):
                    return
                Rj = CHUNKS[j]
                xri = pool.tile(
                    [P, R * W * RI], F32, tag="xri", name="xri", bufs=LOOKAHEAD
                )
                xT[j] = xri
                nc.sync.dma_start(
                    out=xri[:, : Rj * W * RI].rearrange("p (r f) -> p r f", r=Rj),
                    in_=x[:, starts[j] : starts[j] + Rj, :],
                )

            for j in range(LOOKAHEAD):
                load(j)

            for k, Rk in enumerate(CHUNKS):
                r0 = starts[k]
                RPk = Rk // 2
                NH = Rk * W2             # horizontal windows per chunk
                NO = RPk * W2            # output windows per chunk
                NVAL = Rk * W * RI       # f32 values per chunk

                xri = xT.pop(k)[:, :NVAL]

                # squares of the whole chunk in one contiguous ACT op
                sq = pool.tile([P, R * W * RI], F32, tag="sq", name="sq")[:, :NVAL]
                nc.scalar.activation(out=sq, in_=xri, func=ACTF.Square)

                # norm2 = re^2 + im^2 (stride-2 reads, contiguous out)
                nrm = pool.tile([P, R * W2 * 2], F32, tag="nrm", name="nrm")[:, : NH * 2]
                sqp = sq.rearrange("p (n ri) -> p n ri", ri=RI)
                nc.vector.tensor_tensor(
                    out=nrm, in0=sqp[:, :, 0], in1=sqp[:, :, 1], op=OP.add
                )
                nE3 = nrm.rearrange("p (r t w2) -> p r t w2", r=Rk, t=2, w2=W2)
                nE, nO = nE3[:, :, 0, :], nE3[:, :, 1, :]

                # horizontal mask (even/left wins ties) + horizontal max.
                # masks are bf16 0/1.0: bitcast to u8 gives bytes (0x80,0x3f)
                # per true lane -- a pair-broadcast mask with no step-0 AP
                cH = pool.tile([P, R * W2], BF16, tag="cH", name="cH")[:, :NH]
                cH3 = cH.rearrange("p (r w2) -> p r w2", r=Rk, w2=W2)
                nc.vector.tensor_tensor(out=cH3, in0=nE, in1=nO, op=OP.is_ge)
                mH = pool.tile([P, R * W2], F32, tag="mH", name="mH")[:, :NH]
                mH3 = mH.rearrange("p (r w2) -> p r w2", r=Rk, w2=W2)
                nc.vector.tensor_tensor(out=mH3, in0=nE, in1=nO, op=OP.max)

                # horizontal select of (re,im) pairs: pre-fill odd-column
                # candidates (contiguous block), overwrite where even wins
                xp = xri.rearrange(
                    "p (r t w2 ri) -> p r t w2 ri", r=Rk, t=2, w2=W2, ri=RI
                )
                riH = pool.tile([P, R * W2 * RI], F32, tag="riH", name="riH")[:, : NH * RI]
                riH4 = riH.rearrange(
                    "p (r w2 ri) -> p r w2 ri", r=Rk, w2=W2, ri=RI
                )
                nc.scalar.copy(out=riH4, in_=xp[:, :, 1, :, :])
                nc.vector.copy_predicated(
                    out=riH4,
                    mask=cH.bitcast(U8).rearrange(
                        "p (r w2 ri) -> p r w2 ri", r=Rk, w2=W2, ri=RI
                    ),
                    data=xp[:, :, 0, :, :],
                )

                # vertical mask from horizontal maxes (top wins ties)
                mHr = mH.rearrange(
                    "p (rp rt w2) -> p rp rt w2", rp=RPk, rt=2, w2=W2
                )
                cV = pool.tile([P, RP * W2], BF16, tag="cV", name="cV")[:, :NO]
                cV3 = cV.rearrange("p (rp w2) -> p rp w2", rp=RPk, w2=W2)
                nc.vector.tensor_tensor(
                    out=cV3, in0=mHr[:, :, 0, :], in1=mHr[:, :, 1, :], op=OP.is_ge
                )

                # vertical select into the output tile
                riHr = riH.rearrange(
                    "p (rp rt w2 ri) -> p rp rt w2 ri", rp=RPk, rt=2, w2=W2, ri=RI
                )
                outT = pool.tile([P, RP * W2 * RI], F32, tag="outT", name="outT")[:, : NO * RI]
                outT4 = outT.rearrange(
                    "p (rp w2 ri) -> p rp w2 ri", rp=RPk, w2=W2, ri=RI
                )
                nc.scalar.copy(out=outT4, in_=riHr[:, :, 1, :, :])
                nc.vector.copy_predicated(
                    out=outT4,
                    mask=cV.bitcast(U8).rearrange(
                        "p (rp w2 ri) -> p rp w2 ri", rp=RPk, w2=W2, ri=RI
                    ),
                    data=riHr[:, :, 0, :, :],
                )

                load(k + LOOKAHEAD)
                nc.sync.dma_start(
                    out=out[:, r0 // 2 : r0 // 2 + RPk, :],
                    in_=outT.rearrange("p (rp f) -> p rp f", rp=RPk),
                )
    nc.compile()
    return nc


def get_nc() -> bass.Bass:
    if not _NC_CACHE:
        _NC_CACHE.append(_build_nc())
    return _NC_CACHE[0]


def kernel(x: np.ndarray, **run_kwargs) -> np.ndarray:
    nc = get_nc()
    xs = np.asarray(x, dtype=np.float32)
    assert xs.shape == (NCORES * B, RI, C, H, W), xs.shape
    # [16,2,64,H,W] -> [16,64,H,W2,t,2] -> row blocks [16,64,H,t,W2,2]
    xt = xs.transpose(0, 2, 3, 4, 1).reshape(NCORES * B, C, H, W2, 2, RI)
    xt = np.ascontiguousarray(xt.transpose(0, 1, 2, 4, 3, 5))
    in_maps = [
        {"x": xt[B * i : B * (i + 1)].reshape(P, H, W * RI)} for i in range(NCORES)
    ]
    res = bass_utils.run_bass_kernel_spmd(
        nc, in_maps, core_ids=list(range(NCORES)), **run_kwargs
    )
    # per-core [128, HO, WO*2] -> [b, c, HO, WO, ri] -> [b, ri, c, HO, WO]
    outs = [
        res.results[i]["out"].reshape(B, C, HO, WO, RI).transpose(0, 4, 1, 2, 3)
        for i in range(NCORES)
    ]
    out = np.concatenate(outs, axis=0)
    if run_kwargs:
        kernel.last_results = res
    return np.ascontiguousarray(out)
